# revision 1
# baseline (speedup 1.0000x reference)
"""Expert-parallel DeepseekV2 MoE kernel for 8 Trainium2 NeuronCores, v7.

vs v3:
  - ALL inputs in one [128, N] bf16 tensor (per-iteration overhead through
    this exec path is ~29us per argument, so argument count is minimized).
  - Shared m1 computes hs^T directly (wsg/wsu tiles stationary, x^T moving)
    instead of m1-then-PE-transpose: fewer PE ops, fewer DVE copies.

Layout of din columns:
  [ xt_packed (KT*T) | xg slot 0..n (KT*cap each) | ident (128)
  | wsg (KT*ISH) | wsu (KT*ISH) | wsd (ISC*D)
  | slot 0: wg 16*I | wu 16*I | wd IT*D | slot 1: ... ]
Output rows: [ys tile 0..7 | ye slot tiles in order].
"""

import numpy as np
import ml_dtypes

import concourse.bass as bass
import concourse.tile as tile
from concourse import bacc, mybir
from concourse.bass_utils import run_bass_kernel_spmd

T, D = 1024, 2048
E, I = 32, 1408
TOPK = 6
N_GROUP, TOPK_GROUP = 8, 3
ROUTED_SCALE = 2.5
SHARED_I = 2 * I

NCORES = 8
ISH = SHARED_I // NCORES   # 352
KT = D // 128              # 16
IT = I // 128              # 11
ISC = 3
IS_SZ = [128, 128, ISH - 256]
WSLOT = 16 * I + 16 * I + IT * D   # 67584 cols per routed slot

F32 = mybir.dt.float32
BF16 = mybir.dt.bfloat16
SILU = mybir.ActivationFunctionType.Silu
BF = ml_dtypes.bfloat16

_PROGRAM_CACHE = {}


def _col_layout(slot_caps):
    off = {}
    o = 0
    off["xt"] = o; o += KT * T
    off["xg"] = []
    for c in slot_caps:
        off["xg"].append(o); o += KT * c
    off["ident"] = o; o += 128
    off["wsg"] = o; o += KT * ISH
    off["wsu"] = o; o += KT * ISH
    off["wsd"] = o; o += ISC * D
    off["wr"] = []
    for _ in slot_caps:
        off["wr"].append(o); o += WSLOT
    off["total"] = o
    return off


def _build_program(n2, n1):
    nc = bacc.Bacc("TRN2", target_bir_lowering=False, debug=False)

    slot_caps = [256] * n2 + [128] * n1
    ntt_total = sum(c // 128 for c in slot_caps)
    off = _col_layout(slot_caps)

    din = nc.dram_tensor("din", [128, off["total"]], BF16,
                         kind="ExternalInput").ap()
    yo = nc.dram_tensor("yo", [8 + ntt_total, 128, D], BF16,
                        kind="ExternalOutput").ap()

    with tile.TileContext(nc) as tc, \
         tc.tile_pool(name="psum", bufs=8, space="PSUM") as psum, \
         tc.tile_pool(name="shres", bufs=1) as shres, \
         tc.tile_pool(name="hspool", bufs=3) as hspool, \
         tc.tile_pool(name="yspool", bufs=2) as yspool, \
         tc.tile_pool(name="xgpool", bufs=2) as xgpool, \
         tc.tile_pool(name="wpool", bufs=4) as wpool, \
         tc.tile_pool(name="wdpool", bufs=3) as wdpool, \
         tc.tile_pool(name="hgpool", bufs=3) as hgpool, \
         tc.tile_pool(name="hpool", bufs=3) as hpool, \
         tc.tile_pool(name="htpool", bufs=3) as htpool, \
         tc.tile_pool(name="ypool", bufs=3) as ypool:

        xt_sb = shres.tile([128, KT * T], BF16, tag="xt")
        for piece in range(4):
            sl = slice(piece * 4 * T, (piece + 1) * 4 * T)
            nc.sync.dma_start(out=xt_sb[:, sl], in_=din[:, sl])
        wsg_sb = shres.tile([128, KT * ISH], BF16, tag="wsg")
        nc.sync.dma_start(out=wsg_sb[:],
                          in_=din[:, off["wsg"]:off["wsg"] + KT * ISH])
        wsu_sb = shres.tile([128, KT * ISH], BF16, tag="wsu")
        nc.sync.dma_start(out=wsu_sb[:],
                          in_=din[:, off["wsu"]:off["wsu"] + KT * ISH])
        wsd_sb = shres.tile([128, ISC * D], BF16, tag="wsd")
        nc.sync.dma_start(out=wsd_sb[:],
                          in_=din[:, off["wsd"]:off["wsd"] + ISC * D])
        id_sb = shres.tile([128, 128], BF16, tag="ident")
        nc.sync.dma_start(out=id_sb[:],
                          in_=din[:, off["ident"]:off["ident"] + 128])

        hsT_sb = shres.tile([128, ISC, T], BF16, tag="hsT")

        # ---- shared m1: hs^T[is, t] directly (wsg/wsu stationary) ----
        for half in range(2):
            tsl = slice(half * 512, (half + 1) * 512)
            pg = {c: psum.tile([128, 512], F32, tag="ps", name=f"sg{half}_{c}")
                  for c in range(ISC)}
            pu = {c: psum.tile([128, 512], F32, tag="ps", name=f"su{half}_{c}")
                  for c in range(ISC)}
            for kc in range(KT):
                xmov = xt_sb[:, kc * T + half * 512: kc * T + (half + 1) * 512]
                for c in range(ISC):
                    sz = IS_SZ[c]
                    nc.tensor.matmul(
                        pg[c][:sz, :], wsg_sb[:, kc * ISH + c * 128:
                                              kc * ISH + c * 128 + sz],
                        xmov, start=(kc == 0), stop=(kc == KT - 1))
                    nc.tensor.matmul(
                        pu[c][:sz, :], wsu_sb[:, kc * ISH + c * 128:
                                              kc * ISH + c * 128 + sz],
                        xmov, start=(kc == 0), stop=(kc == KT - 1))
            for c in range(ISC):
                sz = IS_SZ[c]
                hsg = hspool.tile([128, 512], BF16, tag="hsg")
                nc.scalar.activation(hsg[:sz, :], pg[c][:sz, :], SILU)
                nc.vector.tensor_mul(hsT_sb[:sz, c, tsl],
                                     pu[c][:sz, :], hsg[:sz, :])

        def m2_piece(tt):
            psy = [psum.tile([128, 512], F32, tag="ps", name=f"sy{tt}_{q}")
                   for q in range(4)]
            for c in range(ISC):
                sz = IS_SZ[c]
                for q in range(4):
                    nc.tensor.matmul(
                        psy[q][:],
                        hsT_sb[:sz, c, tt * 128:(tt + 1) * 128],
                        wsd_sb[:sz, c * D + q * 512: c * D + (q + 1) * 512],
                        start=(c == 0), stop=(c == ISC - 1))
            ysb = yspool.tile([128, D], BF16, tag="ysb")
            for q in range(4):
                nc.vector.tensor_copy(ysb[:, q * 512:(q + 1) * 512], psy[q][:])
            nc.sync.dma_start(out=yo[tt], in_=ysb[:])

        m2_sched = {0: (0, 2), 1: (2, 2), 2: (4, 2), 3: (6, 1), 4: (7, 1)}
        # ---- routed slots ----
        yo_row = 8
        for s, cap in enumerate(slot_caps):
            ntt = cap // 128
            xgo = off["xg"][s]
            soff = off["wr"][s]
            xg_sb = xgpool.tile([128, KT * 256], BF16, tag="xg")
            nc.sync.dma_start(out=xg_sb[:, :KT * cap],
                              in_=din[:, xgo:xgo + KT * cap])

            def xg_lhs(kc, tt):
                return xg_sb[:, kc * cap + tt * 128: kc * cap + (tt + 1) * 128]

            hgs = {}
            hss = {}
            for mi, is_gate in ((0, True), (1, False)):
                moff = soff + mi * 16 * I
                ps = {(tt, j): psum.tile([128, 512], F32, tag="ps",
                                         name=f"p{s}_{mi}_{tt}_{j}")
                      for tt in range(ntt) for j in range(3)}
                for ch in range(4):
                    w_sb = wpool.tile([128, 4 * I], BF16, tag="wst")
                    nc.sync.dma_start(
                        out=w_sb[:],
                        in_=din[:, moff + ch * 4 * I: moff + (ch + 1) * 4 * I])
                    for a in range(4):
                        kc = ch * 4 + a
                        for tt in range(ntt):
                            lhs = xg_lhs(kc, tt)
                            for j in range(3):
                                sz = 512 if j < 2 else I - 1024
                                nc.tensor.matmul(
                                    ps[(tt, j)][:, :sz], lhs,
                                    w_sb[:, a * I + j * 512:
                                         a * I + j * 512 + sz],
                                    start=(kc == 0), stop=(kc == KT - 1))
                for tt in range(ntt):
                    if is_gate:
                        hg = hgpool.tile([128, I], BF16, tag="hg")
                        for j in range(3):
                            sz = 512 if j < 2 else I - 1024
                            nc.scalar.activation(
                                hg[:, j * 512:j * 512 + sz],
                                ps[(tt, j)][:, :sz], SILU)
                        hgs[tt] = hg
                    else:
                        h = hpool.tile([128, I], BF16, tag="h")
                        for j in range(3):
                            sz = 512 if j < 2 else I - 1024
                            nc.vector.tensor_mul(
                                h[:, j * 512:j * 512 + sz],
                                ps[(tt, j)][:, :sz],
                                hgs[tt][:, j * 512:j * 512 + sz])
                        hss[tt] = h

            hts = {}
            for tt in range(ntt):
                ht = htpool.tile([128, IT * 128], BF16, tag="ht")
                for g0, gcnt in ((0, 4), (4, 4), (8, 3)):
                    pst = psum.tile([128, 512], BF16, tag="ps",
                                    name=f"t{s}_{tt}_{g0}")
                    for k in range(gcnt):
                        ic = g0 + k
                        nc.tensor.transpose(
                            pst[:, k * 128:(k + 1) * 128],
                            hss[tt][:, ic * 128:(ic + 1) * 128], id_sb[:])
                    nc.vector.tensor_copy(
                        ht[:, g0 * 128:(g0 + gcnt) * 128],
                        pst[:, :gcnt * 128])
                hts[tt] = ht

            wdoff = soff + 2 * 16 * I
            psy = {(tt, q): psum.tile([128, 512], F32, tag="ps",
                                      name=f"y{s}_{tt}_{q}")
                   for tt in range(ntt) for q in range(4)}
            ichunks = [(0, 2), (2, 2), (4, 2), (6, 2), (8, 2), (10, 1)]
            for i0, cnt in ichunks:
                wd_sb = wdpool.tile([128, 2 * D], BF16, tag="wdst")
                nc.sync.dma_start(
                    out=wd_sb[:, :cnt * D],
                    in_=din[:, wdoff + i0 * D: wdoff + (i0 + cnt) * D])
                for a in range(cnt):
                    i = i0 + a
                    for tt in range(ntt):
                        for q in range(4):
                            nc.tensor.matmul(
                                psy[(tt, q)][:],
                                hts[tt][:, i * 128:(i + 1) * 128],
                                wd_sb[:, a * D + q * 512:
                                      a * D + (q + 1) * 512],
                                start=(i == 0), stop=(i == IT - 1))
            for tt in range(ntt):
                ysb = ypool.tile([128, D], BF16, tag="ye_sb")
                for q in range(4):
                    nc.vector.tensor_copy(ysb[:, q * 512:(q + 1) * 512],
                                          psy[(tt, q)][:])
                nc.sync.dma_start(out=yo[yo_row], in_=ysb[:])
                yo_row += 1
            for _tt in range(*(lambda a, n: (a, a + n))(*m2_sched[s])):
                m2_piece(_tt)

    nc.compile()
    return nc


def get_program(n2, n1):
    key = (n2, n1)
    if key not in _PROGRAM_CACHE:
        _PROGRAM_CACHE[key] = _build_program(n2, n1)
    return _PROGRAM_CACHE[key]


def _route_numpy(x, gate_w, bias):
    logits = x @ gate_w
    scores = 1.0 / (1.0 + np.exp(-logits))
    sc = scores + bias[None, :]
    g = sc.reshape(-1, N_GROUP, E // N_GROUP)
    group_scores = np.sort(g, axis=-1)[..., -2:].sum(-1)
    gidx = np.argsort(-group_scores, axis=-1, kind="stable")[:, :TOPK_GROUP]
    gmask = np.zeros((x.shape[0], N_GROUP), np.bool_)
    np.put_along_axis(gmask, gidx, True, axis=-1)
    emask = np.repeat(gmask, E // N_GROUP, axis=-1)
    masked = np.where(emask, sc, -np.inf)
    topk_idx = np.argsort(-masked, axis=-1, kind="stable")[:, :TOPK]
    w = np.take_along_axis(scores, topk_idx, axis=-1)
    w = w / (w.sum(-1, keepdims=True) + 1e-20)
    return topk_idx, w


def _plan(topk_idx, topk_w):
    flat_e = topk_idx.ravel()
    flat_t = np.repeat(np.arange(topk_idx.shape[0]), TOPK)
    flat_w = (topk_w * ROUTED_SCALE).ravel().astype(np.float32)
    order = np.argsort(flat_e, kind="stable")
    sorted_t = flat_t[order]
    sorted_w = flat_w[order]
    counts = np.bincount(flat_e, minlength=E)
    offsets = np.concatenate([[0], np.cumsum(counts)])

    two_slots, one_slots = [], []
    for e in range(E):
        toks = sorted_t[offsets[e]:offsets[e + 1]]
        ws_ = sorted_w[offsets[e]:offsets[e + 1]]
        n = len(toks)
        if n == 0:
            continue
        pos = 0
        while n - pos > 128:
            two_slots.append((e, toks[pos:pos + 256], ws_[pos:pos + 256]))
            pos += 256
        if n - pos > 0:
            one_slots.append((e, toks[pos:], ws_[pos:]))

    best = None
    for a in range(9):
        for b in range(9):
            for e2 in range(8):
                for e1 in range(8):
                    if a > len(one_slots) or b > len(two_slots):
                        continue
                    t2 = len(two_slots) + a - b + e2
                    t1 = len(one_slots) - a + 2 * b + e1
                    if t2 % NCORES or t1 % NCORES or t2 + t1 == 0:
                        continue
                    cost = 3 * (a + 2 * e2 + e1) + 2 * (b + e2 + e1)
                    if best is None or cost < best[0]:
                        best = (cost, a, b, e2, e1)
    _, a, b, e2, e1 = best
    for _ in range(a):
        one_slots.sort(key=lambda s: len(s[1]))
        two_slots.append(one_slots.pop(0))
    for _ in range(b):
        two_slots.sort(key=lambda s: len(s[1]))
        e, tk, ws_ = two_slots.pop()
        one_slots.append((e, tk[:128], ws_[:128]))
        one_slots.append((e, tk[128:], ws_[128:]))
    empty = (0, np.empty(0, np.int64), np.empty(0, np.float32))
    for _ in range(e2):
        two_slots.append(empty)
    for _ in range(e1):
        one_slots.append(empty)

    n2 = len(two_slots) // NCORES
    n1 = len(one_slots) // NCORES
    per_core = [[] for _ in range(NCORES)]
    for si, s in enumerate(two_slots):
        per_core[si % NCORES].append(s)
    for si, s in enumerate(one_slots):
        per_core[si % NCORES].append(s)
    return per_core, n2, n1


def _pack_k(a):
    m = a.shape[1]
    return np.ascontiguousarray(
        a.reshape(KT, 128, m).transpose(1, 0, 2).reshape(128, KT * m))


def _pack_w_chunks(w):
    """[D, I] -> [128, 16*I]: 4-ktile chunks side by side."""
    return np.ascontiguousarray(
        w.reshape(4, 4, 128, I).transpose(2, 0, 1, 3).reshape(128, 16 * I))


def build_in_maps(inputs):
    x = np.asarray(inputs["hidden_states"], np.float32)
    gate_w = np.asarray(inputs["gate_w"], np.float32)
    bias = np.asarray(inputs["e_score_correction_bias"], np.float32)
    w_gate = np.asarray(inputs["w_gate"], np.float32)
    w_up = np.asarray(inputs["w_up"], np.float32)
    w_down = np.asarray(inputs["w_down"], np.float32)
    ws_gate = np.asarray(inputs["ws_gate"], np.float32)
    ws_up = np.asarray(inputs["ws_up"], np.float32)
    ws_down = np.asarray(inputs["ws_down"], np.float32)

    topk_idx, topk_w = _route_numpy(x, gate_w, bias)
    per_core, n2, n1 = _plan(topk_idx, topk_w)
    slot_caps = [256] * n2 + [128] * n1
    off = _col_layout(slot_caps)

    xt_bf = np.ascontiguousarray(x.T.astype(BF))
    xt_packed = _pack_k(xt_bf)
    wg_bf = w_gate.astype(BF)
    wu_bf = w_up.astype(BF)
    wd_bf = w_down.astype(BF)
    wcache = {}

    def expert_w(e):
        if e not in wcache:
            wcache[e] = np.concatenate([
                _pack_w_chunks(wg_bf[e]),
                _pack_w_chunks(wu_bf[e]),
                wd_bf[e].reshape(IT, 128, D).transpose(1, 0, 2)
                .reshape(128, IT * D)], axis=1)
        return wcache[e]

    wsg_bf = ws_gate.astype(BF)
    wsu_bf = ws_up.astype(BF)
    wsd_bf = ws_down.astype(BF)
    identity = np.eye(128, dtype=BF)

    in_maps = []
    for c in range(NCORES):
        wsd_sl = np.zeros((ISC * 128, D), BF)
        wsd_sl[:ISH] = wsd_bf[c * ISH:(c + 1) * ISH]
        parts = [xt_packed]
        for s, (e, idx, _) in enumerate(per_core[c]):
            cap = slot_caps[s]
            xg = np.zeros((D, cap), BF)
            if len(idx):
                xg[:, :len(idx)] = xt_bf[:, idx]
            parts.append(_pack_k(xg))
        parts.append(identity)
        parts.append(_pack_k(wsg_bf[:, c * ISH:(c + 1) * ISH]))
        parts.append(_pack_k(wsu_bf[:, c * ISH:(c + 1) * ISH]))
        parts.append(wsd_sl.reshape(ISC, 128, D).transpose(1, 0, 2)
                     .reshape(128, ISC * D))
        for s, (e, idx, _) in enumerate(per_core[c]):
            parts.append(expert_w(e))
        din = np.ascontiguousarray(np.concatenate(parts, axis=1))
        assert din.shape[1] == off["total"]
        in_maps.append({"din": din})
    return in_maps, per_core, n2, n1


def kernel(**inputs):
    in_maps, per_core, n2, n1 = build_in_maps(inputs)
    nc = get_program(n2, n1)
    res = run_bass_kernel_spmd(nc, in_maps, core_ids=list(range(NCORES)))

    slot_caps = [256] * n2 + [128] * n1
    out = np.zeros((T, D), np.float32)
    for c in range(NCORES):
        r = res.results[c]["yo"].astype(np.float32)
        out += r[:8].reshape(T, D)
        row = 8
        for s, (e, idx, wv) in enumerate(per_core[c]):
            cap = slot_caps[s]
            ntt = cap // 128
            y = r[row:row + ntt].reshape(cap, D)
            row += ntt
            if len(idx):
                out[idx] += wv[:, None] * y[:len(idx)]
    return out.astype(np.float32)



# revision 3
# speedup vs baseline: 1.0813x; 1.0813x over previous
"""Expert-parallel DeepseekV2 MoE kernel for 8 Trainium2 NeuronCores, v7.

vs v3:
  - ALL inputs in one [128, N] bf16 tensor (per-iteration overhead through
    this exec path is ~29us per argument, so argument count is minimized).
  - Shared m1 computes hs^T directly (wsg/wsu tiles stationary, x^T moving)
    instead of m1-then-PE-transpose: fewer PE ops, fewer DVE copies.

Layout of din columns:
  [ xt_packed (KT*T) | xg slot 0..n (KT*cap each) | ident (128)
  | wsg (KT*ISH) | wsu (KT*ISH) | wsd (ISC*D)
  | slot 0: wg 16*I | wu 16*I | wd IT*D | slot 1: ... ]
Output rows: [ys tile 0..7 | ye slot tiles in order].
"""

import numpy as np
import ml_dtypes

import concourse.bass as bass
import concourse.tile as tile
from concourse import bacc, mybir
from concourse.bass_utils import run_bass_kernel_spmd

T, D = 1024, 2048
E, I = 32, 1408
TOPK = 6
N_GROUP, TOPK_GROUP = 8, 3
ROUTED_SCALE = 2.5
SHARED_I = 2 * I

NCORES = 8
ISH = SHARED_I // NCORES   # 352
KT = D // 128              # 16
IT = I // 128              # 11
ISC = 3
IS_SZ = [128, 128, ISH - 256]
WSLOT = 16 * I + 16 * I + IT * D   # 67584 cols per routed slot

F32 = mybir.dt.float32
BF16 = mybir.dt.bfloat16
SILU = mybir.ActivationFunctionType.Silu
BF = ml_dtypes.bfloat16

_PROGRAM_CACHE = {}


def _col_layout(slot_caps):
    off = {}
    o = 0
    off["xt"] = o; o += KT * T
    off["xg"] = []
    for c in slot_caps:
        off["xg"].append(o); o += KT * c
    off["ident"] = o; o += 128
    off["wsg"] = o; o += KT * ISH
    off["wsu"] = o; o += KT * ISH
    off["wsd"] = o; o += ISC * D
    off["wr"] = []
    for _ in slot_caps:
        off["wr"].append(o); o += WSLOT
    off["total"] = o
    return off


def _build_program(n2, n1, reps=1):
    nc = bacc.Bacc("TRN2", target_bir_lowering=False, debug=False)

    slot_caps = [256] * n2 + [128] * n1
    ntt_total = sum(c // 128 for c in slot_caps)
    off = _col_layout(slot_caps)

    din = nc.dram_tensor("din", [128, off["total"]], BF16,
                         kind="ExternalInput").ap()
    yo = nc.dram_tensor("yo", [8 + ntt_total, 128, D], BF16,
                        kind="ExternalOutput").ap()

    from contextlib import ExitStack
    with tile.TileContext(nc) as tc, \
         tc.tile_pool(name="psum", bufs=8, space="PSUM") as psum, \
         tc.tile_pool(name="shres", bufs=1) as shres, \
         tc.tile_pool(name="hspool", bufs=3) as hspool, \
         tc.tile_pool(name="yspool", bufs=2) as yspool, \
         tc.tile_pool(name="xgpool", bufs=2) as xgpool, \
         tc.tile_pool(name="wpool", bufs=4) as wpool, \
         tc.tile_pool(name="wdpool", bufs=3) as wdpool, \
         tc.tile_pool(name="hgpool", bufs=3) as hgpool, \
         tc.tile_pool(name="hpool", bufs=3) as hpool, \
         tc.tile_pool(name="htpool", bufs=3) as htpool, \
         tc.tile_pool(name="ypool", bufs=3) as ypool, \
         ExitStack() as _rep_ctx:

        if reps > 1:
            _rep_ctx.enter_context(tc.For_i(0, reps, name="rep"))

        xt_sb = shres.tile([128, KT * T], BF16, tag="xt")
        for piece in range(4):
            sl = slice(piece * 4 * T, (piece + 1) * 4 * T)
            nc.sync.dma_start(out=xt_sb[:, sl], in_=din[:, sl])
        wsg_sb = shres.tile([128, KT * ISH], BF16, tag="wsg")
        nc.sync.dma_start(out=wsg_sb[:],
                          in_=din[:, off["wsg"]:off["wsg"] + KT * ISH])
        wsu_sb = shres.tile([128, KT * ISH], BF16, tag="wsu")
        nc.sync.dma_start(out=wsu_sb[:],
                          in_=din[:, off["wsu"]:off["wsu"] + KT * ISH])
        wsd_sb = shres.tile([128, ISC * D], BF16, tag="wsd")
        nc.sync.dma_start(out=wsd_sb[:],
                          in_=din[:, off["wsd"]:off["wsd"] + ISC * D])
        id_sb = shres.tile([128, 128], BF16, tag="ident")
        nc.sync.dma_start(out=id_sb[:],
                          in_=din[:, off["ident"]:off["ident"] + 128])

        hsT_sb = shres.tile([128, ISC, T], BF16, tag="hsT")

        # ---- shared m1: hs^T[is, t] directly (wsg/wsu stationary) ----
        for half in range(2):
            tsl = slice(half * 512, (half + 1) * 512)
            pg = {c: psum.tile([128, 512], F32, tag="ps", name=f"sg{half}_{c}")
                  for c in range(ISC)}
            pu = {c: psum.tile([128, 512], F32, tag="ps", name=f"su{half}_{c}")
                  for c in range(ISC)}
            for kc in range(KT):
                xmov = xt_sb[:, kc * T + half * 512: kc * T + (half + 1) * 512]
                for c in range(ISC):
                    sz = IS_SZ[c]
                    nc.tensor.matmul(
                        pg[c][:sz, :], wsg_sb[:, kc * ISH + c * 128:
                                              kc * ISH + c * 128 + sz],
                        xmov, start=(kc == 0), stop=(kc == KT - 1))
                    nc.tensor.matmul(
                        pu[c][:sz, :], wsu_sb[:, kc * ISH + c * 128:
                                              kc * ISH + c * 128 + sz],
                        xmov, start=(kc == 0), stop=(kc == KT - 1))
            for c in range(ISC):
                sz = IS_SZ[c]
                hsg = hspool.tile([128, 512], BF16, tag="hsg")
                nc.scalar.activation(hsg[:sz, :], pg[c][:sz, :], SILU)
                nc.vector.tensor_mul(hsT_sb[:sz, c, tsl],
                                     pu[c][:sz, :], hsg[:sz, :])

        def m2_piece(tt):
            psy = [psum.tile([128, 512], F32, tag="ps", name=f"sy{tt}_{q}")
                   for q in range(4)]
            for c in range(ISC):
                sz = IS_SZ[c]
                for q in range(4):
                    nc.tensor.matmul(
                        psy[q][:],
                        hsT_sb[:sz, c, tt * 128:(tt + 1) * 128],
                        wsd_sb[:sz, c * D + q * 512: c * D + (q + 1) * 512],
                        start=(c == 0), stop=(c == ISC - 1))
            ysb = yspool.tile([128, D], BF16, tag="ysb")
            for q in range(4):
                nc.vector.tensor_copy(ysb[:, q * 512:(q + 1) * 512], psy[q][:])
            nc.sync.dma_start(out=yo[tt], in_=ysb[:])

        m2_sched = {0: (0, 2), 1: (2, 2), 2: (4, 2), 3: (6, 1), 4: (7, 1)}
        # ---- routed slots ----
        yo_row = 8
        for s, cap in enumerate(slot_caps):
            ntt = cap // 128
            xgo = off["xg"][s]
            soff = off["wr"][s]
            xg_sb = xgpool.tile([128, KT * 256], BF16, tag="xg")
            nc.sync.dma_start(out=xg_sb[:, :KT * cap],
                              in_=din[:, xgo:xgo + KT * cap])

            def xg_lhs(kc, tt):
                return xg_sb[:, kc * cap + tt * 128: kc * cap + (tt + 1) * 128]

            hgs = {}
            hss = {}
            for mi, is_gate in ((0, True), (1, False)):
                moff = soff + mi * 16 * I
                ps = {(tt, j): psum.tile([128, 512], F32, tag="ps",
                                         name=f"p{s}_{mi}_{tt}_{j}")
                      for tt in range(ntt) for j in range(3)}
                for ch in range(4):
                    w_sb = wpool.tile([128, 4 * I], BF16, tag="wst")
                    nc.sync.dma_start(
                        out=w_sb[:],
                        in_=din[:, moff + ch * 4 * I: moff + (ch + 1) * 4 * I])
                    for a in range(4):
                        kc = ch * 4 + a
                        for tt in range(ntt):
                            lhs = xg_lhs(kc, tt)
                            for j in range(3):
                                sz = 512 if j < 2 else I - 1024
                                nc.tensor.matmul(
                                    ps[(tt, j)][:, :sz], lhs,
                                    w_sb[:, a * I + j * 512:
                                         a * I + j * 512 + sz],
                                    start=(kc == 0), stop=(kc == KT - 1))
                for tt in range(ntt):
                    if is_gate:
                        hg = hgpool.tile([128, I], BF16, tag="hg")
                        for j in range(3):
                            sz = 512 if j < 2 else I - 1024
                            nc.scalar.activation(
                                hg[:, j * 512:j * 512 + sz],
                                ps[(tt, j)][:, :sz], SILU)
                        hgs[tt] = hg
                    else:
                        h = hpool.tile([128, I], BF16, tag="h")
                        for j in range(3):
                            sz = 512 if j < 2 else I - 1024
                            nc.vector.tensor_mul(
                                h[:, j * 512:j * 512 + sz],
                                ps[(tt, j)][:, :sz],
                                hgs[tt][:, j * 512:j * 512 + sz])
                        hss[tt] = h

            hts = {}
            for tt in range(ntt):
                ht = htpool.tile([128, IT * 128], BF16, tag="ht")
                for g0, gcnt in ((0, 4), (4, 4), (8, 3)):
                    pst = psum.tile([128, 512], BF16, tag="ps",
                                    name=f"t{s}_{tt}_{g0}")
                    for k in range(gcnt):
                        ic = g0 + k
                        nc.tensor.transpose(
                            pst[:, k * 128:(k + 1) * 128],
                            hss[tt][:, ic * 128:(ic + 1) * 128], id_sb[:])
                    nc.vector.tensor_copy(
                        ht[:, g0 * 128:(g0 + gcnt) * 128],
                        pst[:, :gcnt * 128])
                hts[tt] = ht

            wdoff = soff + 2 * 16 * I
            psy = {(tt, q): psum.tile([128, 512], F32, tag="ps",
                                      name=f"y{s}_{tt}_{q}")
                   for tt in range(ntt) for q in range(4)}
            ichunks = [(0, 2), (2, 2), (4, 2), (6, 2), (8, 2), (10, 1)]
            for i0, cnt in ichunks:
                wd_sb = wdpool.tile([128, 2 * D], BF16, tag="wdst")
                nc.sync.dma_start(
                    out=wd_sb[:, :cnt * D],
                    in_=din[:, wdoff + i0 * D: wdoff + (i0 + cnt) * D])
                for a in range(cnt):
                    i = i0 + a
                    for tt in range(ntt):
                        for q in range(4):
                            nc.tensor.matmul(
                                psy[(tt, q)][:],
                                hts[tt][:, i * 128:(i + 1) * 128],
                                wd_sb[:, a * D + q * 512:
                                      a * D + (q + 1) * 512],
                                start=(i == 0), stop=(i == IT - 1))
            for tt in range(ntt):
                ysb = ypool.tile([128, D], BF16, tag="ye_sb")
                for q in range(4):
                    nc.vector.tensor_copy(ysb[:, q * 512:(q + 1) * 512],
                                          psy[(tt, q)][:])
                nc.sync.dma_start(out=yo[yo_row], in_=ysb[:])
                yo_row += 1
            for _tt in range(*(lambda a, n: (a, a + n))(*m2_sched[s])):
                m2_piece(_tt)

    nc.compile()
    return nc


def get_program(n2, n1, reps=1):
    key = (n2, n1, reps)
    if key not in _PROGRAM_CACHE:
        _PROGRAM_CACHE[key] = _build_program(n2, n1, reps)
    return _PROGRAM_CACHE[key]


def _route_numpy(x, gate_w, bias):
    logits = x @ gate_w
    scores = 1.0 / (1.0 + np.exp(-logits))
    sc = scores + bias[None, :]
    g = sc.reshape(-1, N_GROUP, E // N_GROUP)
    group_scores = np.sort(g, axis=-1)[..., -2:].sum(-1)
    gidx = np.argsort(-group_scores, axis=-1, kind="stable")[:, :TOPK_GROUP]
    gmask = np.zeros((x.shape[0], N_GROUP), np.bool_)
    np.put_along_axis(gmask, gidx, True, axis=-1)
    emask = np.repeat(gmask, E // N_GROUP, axis=-1)
    masked = np.where(emask, sc, -np.inf)
    topk_idx = np.argsort(-masked, axis=-1, kind="stable")[:, :TOPK]
    w = np.take_along_axis(scores, topk_idx, axis=-1)
    w = w / (w.sum(-1, keepdims=True) + 1e-20)
    return topk_idx, w


def _plan(topk_idx, topk_w):
    flat_e = topk_idx.ravel()
    flat_t = np.repeat(np.arange(topk_idx.shape[0]), TOPK)
    flat_w = (topk_w * ROUTED_SCALE).ravel().astype(np.float32)
    order = np.argsort(flat_e, kind="stable")
    sorted_t = flat_t[order]
    sorted_w = flat_w[order]
    counts = np.bincount(flat_e, minlength=E)
    offsets = np.concatenate([[0], np.cumsum(counts)])

    two_slots, one_slots = [], []
    for e in range(E):
        toks = sorted_t[offsets[e]:offsets[e + 1]]
        ws_ = sorted_w[offsets[e]:offsets[e + 1]]
        n = len(toks)
        if n == 0:
            continue
        pos = 0
        while n - pos > 128:
            two_slots.append((e, toks[pos:pos + 256], ws_[pos:pos + 256]))
            pos += 256
        if n - pos > 0:
            one_slots.append((e, toks[pos:], ws_[pos:]))

    best = None
    for a in range(9):
        for b in range(9):
            for e2 in range(8):
                for e1 in range(8):
                    if a > len(one_slots) or b > len(two_slots):
                        continue
                    t2 = len(two_slots) + a - b + e2
                    t1 = len(one_slots) - a + 2 * b + e1
                    if t2 % NCORES or t1 % NCORES or t2 + t1 == 0:
                        continue
                    cost = 3 * (a + 2 * e2 + e1) + 2 * (b + e2 + e1)
                    if best is None or cost < best[0]:
                        best = (cost, a, b, e2, e1)
    _, a, b, e2, e1 = best
    for _ in range(a):
        one_slots.sort(key=lambda s: len(s[1]))
        two_slots.append(one_slots.pop(0))
    for _ in range(b):
        two_slots.sort(key=lambda s: len(s[1]))
        e, tk, ws_ = two_slots.pop()
        one_slots.append((e, tk[:128], ws_[:128]))
        one_slots.append((e, tk[128:], ws_[128:]))
    empty = (0, np.empty(0, np.int64), np.empty(0, np.float32))
    for _ in range(e2):
        two_slots.append(empty)
    for _ in range(e1):
        one_slots.append(empty)

    n2 = len(two_slots) // NCORES
    n1 = len(one_slots) // NCORES
    per_core = [[] for _ in range(NCORES)]
    for si, s in enumerate(two_slots):
        per_core[si % NCORES].append(s)
    for si, s in enumerate(one_slots):
        per_core[si % NCORES].append(s)
    return per_core, n2, n1


def _pack_k(a):
    m = a.shape[1]
    return np.ascontiguousarray(
        a.reshape(KT, 128, m).transpose(1, 0, 2).reshape(128, KT * m))


def _pack_w_chunks(w):
    """[D, I] -> [128, 16*I]: 4-ktile chunks side by side."""
    return np.ascontiguousarray(
        w.reshape(4, 4, 128, I).transpose(2, 0, 1, 3).reshape(128, 16 * I))


def build_in_maps(inputs):
    x = np.asarray(inputs["hidden_states"], np.float32)
    gate_w = np.asarray(inputs["gate_w"], np.float32)
    bias = np.asarray(inputs["e_score_correction_bias"], np.float32)
    w_gate = np.asarray(inputs["w_gate"], np.float32)
    w_up = np.asarray(inputs["w_up"], np.float32)
    w_down = np.asarray(inputs["w_down"], np.float32)
    ws_gate = np.asarray(inputs["ws_gate"], np.float32)
    ws_up = np.asarray(inputs["ws_up"], np.float32)
    ws_down = np.asarray(inputs["ws_down"], np.float32)

    topk_idx, topk_w = _route_numpy(x, gate_w, bias)
    per_core, n2, n1 = _plan(topk_idx, topk_w)
    slot_caps = [256] * n2 + [128] * n1
    off = _col_layout(slot_caps)

    xt_bf = np.ascontiguousarray(x.T.astype(BF))
    xt_packed = _pack_k(xt_bf)
    wg_bf = w_gate.astype(BF)
    wu_bf = w_up.astype(BF)
    wd_bf = w_down.astype(BF)
    wcache = {}

    def expert_w(e):
        if e not in wcache:
            wcache[e] = np.concatenate([
                _pack_w_chunks(wg_bf[e]),
                _pack_w_chunks(wu_bf[e]),
                wd_bf[e].reshape(IT, 128, D).transpose(1, 0, 2)
                .reshape(128, IT * D)], axis=1)
        return wcache[e]

    wsg_bf = ws_gate.astype(BF)
    wsu_bf = ws_up.astype(BF)
    wsd_bf = ws_down.astype(BF)
    identity = np.eye(128, dtype=BF)

    in_maps = []
    for c in range(NCORES):
        wsd_sl = np.zeros((ISC * 128, D), BF)
        wsd_sl[:ISH] = wsd_bf[c * ISH:(c + 1) * ISH]
        parts = [xt_packed]
        for s, (e, idx, _) in enumerate(per_core[c]):
            cap = slot_caps[s]
            xg = np.zeros((D, cap), BF)
            if len(idx):
                xg[:, :len(idx)] = xt_bf[:, idx]
            parts.append(_pack_k(xg))
        parts.append(identity)
        parts.append(_pack_k(wsg_bf[:, c * ISH:(c + 1) * ISH]))
        parts.append(_pack_k(wsu_bf[:, c * ISH:(c + 1) * ISH]))
        parts.append(wsd_sl.reshape(ISC, 128, D).transpose(1, 0, 2)
                     .reshape(128, ISC * D))
        for s, (e, idx, _) in enumerate(per_core[c]):
            parts.append(expert_w(e))
        din = np.ascontiguousarray(np.concatenate(parts, axis=1))
        assert din.shape[1] == off["total"]
        in_maps.append({"din": din})
    return in_maps, per_core, n2, n1


def kernel(**inputs):
    in_maps, per_core, n2, n1 = build_in_maps(inputs)
    nc = get_program(n2, n1)
    res = run_bass_kernel_spmd(nc, in_maps, core_ids=list(range(NCORES)))

    slot_caps = [256] * n2 + [128] * n1
    out = np.zeros((T, D), np.float32)
    for c in range(NCORES):
        r = res.results[c]["yo"].astype(np.float32)
        out += r[:8].reshape(T, D)
        row = 8
        for s, (e, idx, wv) in enumerate(per_core[c]):
            cap = slot_caps[s]
            ntt = cap // 128
            y = r[row:row + ntt].reshape(cap, D)
            row += ntt
            if len(idx):
                out[idx] += wv[:, None] * y[:len(idx)]
    return out.astype(np.float32)



# revision 41
# speedup vs baseline: 1.0826x; 1.0012x over previous
"""Expert-parallel DeepseekV2 MoE kernel for 8 Trainium2 NeuronCores, v7.

vs v3:
  - ALL inputs in one [128, N] bf16 tensor (per-iteration overhead through
    this exec path is ~29us per argument, so argument count is minimized).
  - Shared m1 computes hs^T directly (wsg/wsu tiles stationary, x^T moving)
    instead of m1-then-PE-transpose: fewer PE ops, fewer DVE copies.

Layout of din columns:
  [ xt_packed (KT*T) | xg slot 0..n (KT*cap each) | ident (128)
  | wsg (KT*ISH) | wsu (KT*ISH) | wsd (ISC*D)
  | slot 0: wg 16*I | wu 16*I | wd IT*D | slot 1: ... ]
Output rows: [ys tile 0..7 | ye slot tiles in order].
"""

import numpy as np
import ml_dtypes

import concourse.bass as bass
import concourse.tile as tile
from concourse import bacc, mybir
from concourse.bass_utils import run_bass_kernel_spmd

T, D = 1024, 2048
E, I = 32, 1408
TOPK = 6
N_GROUP, TOPK_GROUP = 8, 3
ROUTED_SCALE = 2.5
SHARED_I = 2 * I

NCORES = 8
ISH = SHARED_I // NCORES   # 352
KT = D // 128              # 16
IT = I // 128              # 11
ISC = 3
IS_SZ = [128, 128, ISH - 256]
WSLOT = 16 * I + 16 * I + IT * D   # 67584 cols per routed slot

F32 = mybir.dt.float32
BF16 = mybir.dt.bfloat16
SILU = mybir.ActivationFunctionType.Silu
COPYF = mybir.ActivationFunctionType.Copy
BF = ml_dtypes.bfloat16

_PROGRAM_CACHE = {}


def _col_layout(slot_caps):
    off = {}
    o = 0
    off["xt"] = o; o += KT * T
    off["xg"] = []
    for c in slot_caps:
        off["xg"].append(o); o += KT * c
    off["ident"] = o; o += 128
    off["wsg"] = o; o += KT * ISH
    off["wsu"] = o; o += KT * ISH
    off["wsd"] = o; o += ISC * D
    off["wr"] = []
    for _ in slot_caps:
        off["wr"].append(o); o += WSLOT
    off["total"] = o
    return off


def _build_program(n2, n1, reps=1):
    nc = bacc.Bacc("TRN2", target_bir_lowering=False, debug=False)

    slot_caps = [256] * n2 + [128] * n1
    ntt_total = sum(c // 128 for c in slot_caps)
    off = _col_layout(slot_caps)
    order = _slot_order(n2, n1)

    din = nc.dram_tensor("din", [128, off["total"]], BF16,
                         kind="ExternalInput").ap()
    yo = nc.dram_tensor("yo", [8 + ntt_total, 128, D], BF16,
                        kind="ExternalOutput").ap()

    from contextlib import ExitStack
    with tile.TileContext(nc) as tc, \
         tc.tile_pool(name="psum", bufs=8, space="PSUM") as psum, \
         tc.tile_pool(name="shres", bufs=1) as shres, \
         tc.tile_pool(name="hspool", bufs=3) as hspool, \
         tc.tile_pool(name="yspool", bufs=2) as yspool, \
         tc.tile_pool(name="xgpool", bufs=2) as xgpool, \
         tc.tile_pool(name="wpool", bufs=5) as wpool, \
         tc.tile_pool(name="wdpool", bufs=3) as wdpool, \
         tc.tile_pool(name="hgpool", bufs=2) as hgpool, \
         tc.tile_pool(name="hpool", bufs=2) as hpool, \
         tc.tile_pool(name="htpool", bufs=2) as htpool, \
         tc.tile_pool(name="ypool", bufs=2) as ypool, \
         ExitStack() as _rep_ctx:

        if reps > 1:
            _rep_ctx.enter_context(tc.For_i(0, reps, name="rep"))

        # Interleave xt / wsg / wsu chunks in shared-m1 consumption order
        # (kc ascending) so the first matmul waits on ~1.4MB, not ~8.4MB.
        xt0_sb = shres.tile([128, KT * 512], BF16, tag="xt0")
        xt1_sb = shres.tile([128, KT * 512], BF16, tag="xt1")
        wsg_sb = shres.tile([128, KT * ISH], BF16, tag="wsg")
        wsu_sb = shres.tile([128, KT * ISH], BF16, tag="wsu")
        HW = KT * 512
        _ingrp = [(0, 1), (1, 1), (2, 2), (4, 4), (8, 8)]
        for g0, gn in _ingrp:
            nc.sync.dma_start(out=wsg_sb[:, g0 * ISH:(g0 + gn) * ISH],
                              in_=din[:, off["wsg"] + g0 * ISH:
                                       off["wsg"] + (g0 + gn) * ISH])
            nc.sync.dma_start(out=xt0_sb[:, g0 * 512:(g0 + gn) * 512],
                              in_=din[:, g0 * 512:(g0 + gn) * 512])
            nc.sync.dma_start(out=wsu_sb[:, g0 * ISH:(g0 + gn) * ISH],
                              in_=din[:, off["wsu"] + g0 * ISH:
                                       off["wsu"] + (g0 + gn) * ISH])
            if g0 == 4:
                nc.sync.dma_start(out=xt1_sb[:, :HW // 2],
                                  in_=din[:, HW:HW + HW // 2])
        nc.sync.dma_start(out=xt1_sb[:, HW // 2:],
                          in_=din[:, HW + HW // 2:2 * HW])
        wsd_sb = shres.tile([128, ISC * D], BF16, tag="wsd")
        nc.sync.dma_start(out=wsd_sb[:],
                          in_=din[:, off["wsd"]:off["wsd"] + ISC * D])
        id_sb = shres.tile([128, 128], BF16, tag="ident")
        nc.sync.dma_start(out=id_sb[:],
                          in_=din[:, off["ident"]:off["ident"] + 128])

        hsT_sb = shres.tile([128, ISC, T], BF16, tag="hsT")

        def m1_cmul(c, half, pg, pu):
            sz = IS_SZ[c]
            tsl = slice(half * 512, (half + 1) * 512)
            hsg = hspool.tile([128, 512], BF16, tag="hsg")
            nc.scalar.activation(hsg[:sz, :], pg[:sz, :], SILU)
            nc.vector.tensor_mul(hsT_sb[:sz, c, tsl], pu[:sz, :], hsg[:sz, :])

        # shared m1, half 0: streamed kc-outer at the head (consumes input
        # chunks as they arrive).
        def m1_half_stream(half):
            pg = {c: psum.tile([128, 512], F32, tag="ps", name=f"sg{half}_{c}")
                  for c in range(ISC)}
            pu = {c: psum.tile([128, 512], F32, tag="ps", name=f"su{half}_{c}")
                  for c in range(ISC)}
            xts = xt0_sb if half == 0 else xt1_sb
            for kc in range(KT):
                xmov = xts[:, kc * 512:(kc + 1) * 512]
                for c in range(ISC):
                    sz = IS_SZ[c]
                    nc.tensor.matmul(
                        pg[c][:sz, :], wsg_sb[:, kc * ISH + c * 128:
                                              kc * ISH + c * 128 + sz],
                        xmov, start=(kc == 0), stop=(kc == KT - 1))
                    nc.tensor.matmul(
                        pu[c][:sz, :], wsu_sb[:, kc * ISH + c * 128:
                                              kc * ISH + c * 128 + sz],
                        xmov, start=(kc == 0), stop=(kc == KT - 1))
            for c in range(ISC):
                m1_cmul(c, half, pg[c], pu[c])

        # shared m2 half-piece: 2 of 4 q-quarters of one 128-token tile.
        # 6 matmuls, 2 psum banks, ~2.6us of DMA-free PE work.
        def m2_half(tt, qp):
            qs = (0, 1) if qp == 0 else (2, 3)
            psy = {q: psum.tile([128, 512], F32, tag="ps", name=f"sy{tt}_{q}")
                   for q in qs}
            for c in range(ISC):
                sz = IS_SZ[c]
                for q in qs:
                    nc.tensor.matmul(
                        psy[q][:],
                        hsT_sb[:sz, c, tt * 128:(tt + 1) * 128],
                        wsd_sb[:sz, c * D + q * 512: c * D + (q + 1) * 512],
                        start=(c == 0), stop=(c == ISC - 1))
            ysb = yspool.tile([128, 1024], BF16, tag="ysb")
            for k, q in enumerate(qs):
                nc.vector.tensor_copy(ysb[:, k * 512:(k + 1) * 512],
                                      psy[q][:])
            nc.sync.dma_start(out=yo[tt, :, qp * 1024:(qp + 1) * 1024],
                              in_=ysb[:])

        m1_half_stream(0)

        # shared m1 half-1 sub-units: 16 matmuls (3.4us) of DMA-free PE work
        # each; g and u of the same c-block may sit at different hooks (the
        # psum accumulator is carried between them).
        m1_pg = {}

        def m1_sub_g(c):
            sz = IS_SZ[c]
            pg = psum.tile([128, 512], F32, tag="ps", name=f"sg1_{c}")
            for kc in range(KT):
                nc.tensor.matmul(
                    pg[:sz, :], wsg_sb[:, kc * ISH + c * 128:
                                       kc * ISH + c * 128 + sz],
                    xt1_sb[:, kc * 512:(kc + 1) * 512],
                    start=(kc == 0), stop=(kc == KT - 1))
            m1_pg[c] = pg

        def m1_sub_u(c):
            sz = IS_SZ[c]
            pu = psum.tile([128, 512], F32, tag="ps", name=f"su1_{c}")
            for kc in range(KT):
                nc.tensor.matmul(
                    pu[:sz, :], wsu_sb[:, kc * ISH + c * 128:
                                       kc * ISH + c * 128 + sz],
                    xt1_sb[:, kc * 512:(kc + 1) * 512],
                    start=(kc == 0), stop=(kc == KT - 1))
            m1_cmul(c, 1, m1_pg.pop(c), pu)

        order = _slot_order(n2, n1)

        # ---- filler plan: DMA-free shared-expert units at the measured
        # stall points. Units: ("g",c)/("u",c) m1 sub-units (3.4us),
        # ("h",tt,qp) m2 half-pieces (1.3us), ("p",tt) full pieces (2.6us).
        fill_plan = {}
        if n2 == 3 and n1 == 2:
            fill_plan = {
                (0, "head"): [("g", 0), ("u", 0)],
                (0, "uend"): [("h", 0, 0), ("h", 0, 1)],
                (1, "gstart"): [("h", 1, 0)],
                (1, "uend"): [("h", 1, 1), ("h", 2, 0)],
                (2, "gstart"): [("h", 2, 1)],
                (2, "uend"): [("h", 3, 0), ("h", 3, 1)],
                (3, "gstart"): [("g", 1)],
                (3, "gmid"): [("u", 1)],
                (3, "gend"): [("g", 2), ("u", 2)],
                (3, "umid"): [("p", 4)],
                (4, "gstart"): [("p", 7)],
                (4, "gmid"): [("p", 5)],
                (4, "umid"): [("p", 6)],
            }
        else:
            fill_plan[(0, "head")] = [("g", c) for c in range(ISC)] + \
                                     [("u", c) for c in range(ISC)]
            hooks = []
            for pos, s in enumerate(order):
                if slot_caps[s] == 128:
                    hooks += [(pos, "gmid"), (pos, "gend"), (pos, "mmid")]
                elif pos > 0:
                    hooks.append((pos, "gstart"))
            if not hooks:
                hooks = [(len(order) - 1, "m2start")]
            for tt in range(8):
                fill_plan.setdefault(hooks[tt % len(hooks)], []).append(
                    ("p", tt))

        def emit_fill(pos, phase):
            for u in fill_plan.get((pos, phase), ()):
                if u[0] == "g":
                    m1_sub_g(u[1])
                elif u[0] == "u":
                    m1_sub_u(u[1])
                elif u[0] == "h":
                    m2_half(u[1], u[2])
                else:
                    m2_half(u[1], 0)
                    m2_half(u[1], 1)

        emit_fill(0, "head")

        next_yo_row = [8]
        for pos, s in enumerate(order):
            cap = slot_caps[s]
            ntt = cap // 128
            xgo = off["xg"][s]
            soff = off["wr"][s]
            xg_sb = xgpool.tile([128, KT * 256], BF16, tag="xg")
            nc.sync.dma_start(out=xg_sb[:, :KT * cap],
                              in_=din[:, xgo:xgo + KT * cap])

            def xg_lhs(kc, tt):
                return xg_sb[:, kc * cap + tt * 128: kc * cap + (tt + 1) * 128]

            emit_fill(pos, "gstart")
            hgs = {}
            hss = {}
            for mi, is_gate in ((0, True), (1, False)):
                moff = soff + mi * 16 * I
                ps = {(tt, j): psum.tile([128, 512], F32, tag="ps",
                                         name=f"p{s}_{mi}_{tt}_{j}")
                      for tt in range(ntt) for j in range(3)}
                for ch in range(4):
                    w_sb = wpool.tile([128, 4 * I], BF16, tag="wst",
                                      name="w_sb")
                    nc.sync.dma_start(
                        out=w_sb[:],
                        in_=din[:, moff + ch * 4 * I: moff + (ch + 1) * 4 * I])
                    for a in range(4):
                        kc = ch * 4 + a
                        for tt in range(ntt):
                            lhs = xg_lhs(kc, tt)
                            for j in range(3):
                                sz = 512 if j < 2 else I - 1024
                                nc.tensor.matmul(
                                    ps[(tt, j)][:, :sz], lhs,
                                    w_sb[:, a * I + j * 512:
                                         a * I + j * 512 + sz],
                                    start=(kc == 0), stop=(kc == KT - 1))
                    if ch == 1:
                        emit_fill(pos, "gmid" if is_gate else "umid")
                if is_gate:
                    for tt in range(ntt):
                        hgs[tt] = hgpool.tile([128, I], BF16, tag="hg",
                                              name=f"hg{tt}")
                else:
                    for tt in range(ntt):
                        hss[tt] = hpool.tile([128, I], BF16, tag="h",
                                             name=f"h{tt}")
                for j in range(3):
                    sz = 512 if j < 2 else I - 1024
                    for tt in range(ntt):
                        if is_gate:
                            nc.scalar.activation(
                                hgs[tt][:, j * 512:j * 512 + sz],
                                ps[(tt, j)][:, :sz], SILU)
                        else:
                            nc.vector.tensor_mul(
                                hss[tt][:, j * 512:j * 512 + sz],
                                ps[(tt, j)][:, :sz],
                                hgs[tt][:, j * 512:j * 512 + sz])
                emit_fill(pos, "gend" if is_gate else "uend")

            hts = {tt: htpool.tile([128, IT * 128], BF16, tag="ht",
                                   name=f"ht{tt}")
                   for tt in range(ntt)}
            for g0, gcnt in ((0, 4), (4, 4), (8, 3)):
                for tt in range(ntt):
                    pst = psum.tile([128, 512], BF16, tag="ps",
                                    name=f"t{s}_{tt}_{g0}")
                    for k in range(gcnt):
                        ic = g0 + k
                        nc.tensor.transpose(
                            pst[:, k * 128:(k + 1) * 128],
                            hss[tt][:, ic * 128:(ic + 1) * 128], id_sb[:])
                    nc.vector.tensor_copy(
                        hts[tt][:, g0 * 128:(g0 + gcnt) * 128],
                        pst[:, :gcnt * 128])

            emit_fill(pos, "m2start")
            wdoff = soff + 2 * 16 * I
            psy = {(tt, q): psum.tile([128, 512], F32, tag="ps",
                                      name=f"y{s}_{tt}_{q}")
                   for tt in range(ntt) for q in range(4)}
            ichunks = [(0, 2), (2, 2), (4, 2), (6, 2), (8, 2), (10, 1)]
            for nch, (i0, cnt) in enumerate(ichunks):
                wd_sb = wdpool.tile([128, 2 * D], BF16, tag="wdst")
                nc.sync.dma_start(
                    out=wd_sb[:, :cnt * D],
                    in_=din[:, wdoff + i0 * D: wdoff + (i0 + cnt) * D])
                for a in range(cnt):
                    i = i0 + a
                    for tt in range(ntt):
                        for q in range(4):
                            nc.tensor.matmul(
                                psy[(tt, q)][:],
                                hts[tt][:, i * 128:(i + 1) * 128],
                                wd_sb[:, a * D + q * 512:
                                      a * D + (q + 1) * 512],
                                start=(i == 0), stop=(i == IT - 1))
                if nch == 2:
                    emit_fill(pos, "mmid")
            for tt in range(ntt):
                ysb = ypool.tile([128, D], BF16, tag="ye_sb")
                for q in range(4):
                    nc.vector.tensor_copy(ysb[:, q * 512:(q + 1) * 512],
                                          psy[(tt, q)][:])
                nc.sync.dma_start(out=yo[next_yo_row[0]], in_=ysb[:])
                next_yo_row[0] += 1

    nc.compile()
    return nc


def _slot_order(n2, n1):
    """2-tile slots first: their weight streams are lighter than their PE
    demand, so DMA banks lookahead into the weight pools; the 1-tile slots
    (DMA-heavy, PE-light) then run cushioned by that lead plus filler."""
    return list(range(n2 + n1))


def get_program(n2, n1, reps=1):
    key = (n2, n1, reps)
    if key not in _PROGRAM_CACHE:
        _PROGRAM_CACHE[key] = _build_program(n2, n1, reps)
    return _PROGRAM_CACHE[key]


def _route_numpy(x, gate_w, bias):
    logits = x @ gate_w
    scores = 1.0 / (1.0 + np.exp(-logits))
    sc = scores + bias[None, :]
    g = sc.reshape(-1, N_GROUP, E // N_GROUP)
    group_scores = np.sort(g, axis=-1)[..., -2:].sum(-1)
    gidx = np.argsort(-group_scores, axis=-1, kind="stable")[:, :TOPK_GROUP]
    gmask = np.zeros((x.shape[0], N_GROUP), np.bool_)
    np.put_along_axis(gmask, gidx, True, axis=-1)
    emask = np.repeat(gmask, E // N_GROUP, axis=-1)
    masked = np.where(emask, sc, -np.inf)
    topk_idx = np.argsort(-masked, axis=-1, kind="stable")[:, :TOPK]
    w = np.take_along_axis(scores, topk_idx, axis=-1)
    w = w / (w.sum(-1, keepdims=True) + 1e-20)
    return topk_idx, w


def _plan(topk_idx, topk_w):
    flat_e = topk_idx.ravel()
    flat_t = np.repeat(np.arange(topk_idx.shape[0]), TOPK)
    flat_w = (topk_w * ROUTED_SCALE).ravel().astype(np.float32)
    order = np.argsort(flat_e, kind="stable")
    sorted_t = flat_t[order]
    sorted_w = flat_w[order]
    counts = np.bincount(flat_e, minlength=E)
    offsets = np.concatenate([[0], np.cumsum(counts)])

    two_slots, one_slots = [], []
    for e in range(E):
        toks = sorted_t[offsets[e]:offsets[e + 1]]
        ws_ = sorted_w[offsets[e]:offsets[e + 1]]
        n = len(toks)
        if n == 0:
            continue
        pos = 0
        while n - pos > 128:
            two_slots.append((e, toks[pos:pos + 256], ws_[pos:pos + 256]))
            pos += 256
        if n - pos > 0:
            one_slots.append((e, toks[pos:], ws_[pos:]))

    best = None
    for a in range(9):
        for b in range(9):
            for e2 in range(8):
                for e1 in range(8):
                    if a > len(one_slots) or b > len(two_slots):
                        continue
                    t2 = len(two_slots) + a - b + e2
                    t1 = len(one_slots) - a + 2 * b + e1
                    if t2 % NCORES or t1 % NCORES or t2 + t1 == 0:
                        continue
                    cost = 3 * (a + 2 * e2 + e1) + 2 * (b + e2 + e1)
                    if best is None or cost < best[0]:
                        best = (cost, a, b, e2, e1)
    _, a, b, e2, e1 = best
    for _ in range(a):
        one_slots.sort(key=lambda s: len(s[1]))
        two_slots.append(one_slots.pop(0))
    for _ in range(b):
        two_slots.sort(key=lambda s: len(s[1]))
        e, tk, ws_ = two_slots.pop()
        one_slots.append((e, tk[:128], ws_[:128]))
        one_slots.append((e, tk[128:], ws_[128:]))
    empty = (0, np.empty(0, np.int64), np.empty(0, np.float32))
    for _ in range(e2):
        two_slots.append(empty)
    for _ in range(e1):
        one_slots.append(empty)

    n2 = len(two_slots) // NCORES
    n1 = len(one_slots) // NCORES
    per_core = [[] for _ in range(NCORES)]
    for si, s in enumerate(two_slots):
        per_core[si % NCORES].append(s)
    for si, s in enumerate(one_slots):
        per_core[si % NCORES].append(s)
    return per_core, n2, n1


def _pack_k(a):
    m = a.shape[1]
    return np.ascontiguousarray(
        a.reshape(KT, 128, m).transpose(1, 0, 2).reshape(128, KT * m))


def _pack_w_chunks(w):
    """[D, I] -> [128, 16*I]: 4-ktile chunks side by side."""
    return np.ascontiguousarray(
        w.reshape(4, 4, 128, I).transpose(2, 0, 1, 3).reshape(128, 16 * I))


def build_in_maps(inputs):
    x = np.asarray(inputs["hidden_states"], np.float32)
    gate_w = np.asarray(inputs["gate_w"], np.float32)
    bias = np.asarray(inputs["e_score_correction_bias"], np.float32)
    w_gate = np.asarray(inputs["w_gate"], np.float32)
    w_up = np.asarray(inputs["w_up"], np.float32)
    w_down = np.asarray(inputs["w_down"], np.float32)
    ws_gate = np.asarray(inputs["ws_gate"], np.float32)
    ws_up = np.asarray(inputs["ws_up"], np.float32)
    ws_down = np.asarray(inputs["ws_down"], np.float32)

    topk_idx, topk_w = _route_numpy(x, gate_w, bias)
    per_core, n2, n1 = _plan(topk_idx, topk_w)
    slot_caps = [256] * n2 + [128] * n1
    off = _col_layout(slot_caps)

    xt_bf = np.ascontiguousarray(x.T.astype(BF))
    _a3 = xt_bf.reshape(KT, 128, T)
    xt_packed = np.concatenate(
        [np.ascontiguousarray(_a3[:, :, h * 512:(h + 1) * 512]
                              .transpose(1, 0, 2).reshape(128, KT * 512))
         for h in (0, 1)], axis=1)
    wg_bf = w_gate.astype(BF)
    wu_bf = w_up.astype(BF)
    wd_bf = w_down.astype(BF)
    wcache = {}

    def expert_w(e):
        if e not in wcache:
            wcache[e] = np.concatenate([
                _pack_w_chunks(wg_bf[e]),
                _pack_w_chunks(wu_bf[e]),
                wd_bf[e].reshape(IT, 128, D).transpose(1, 0, 2)
                .reshape(128, IT * D)], axis=1)
        return wcache[e]

    wsg_bf = ws_gate.astype(BF)
    wsu_bf = ws_up.astype(BF)
    wsd_bf = ws_down.astype(BF)
    identity = np.eye(128, dtype=BF)

    in_maps = []
    for c in range(NCORES):
        wsd_sl = np.zeros((ISC * 128, D), BF)
        wsd_sl[:ISH] = wsd_bf[c * ISH:(c + 1) * ISH]
        parts = [xt_packed]
        for s, (e, idx, _) in enumerate(per_core[c]):
            cap = slot_caps[s]
            xg = np.zeros((D, cap), BF)
            if len(idx):
                xg[:, :len(idx)] = xt_bf[:, idx]
            parts.append(_pack_k(xg))
        parts.append(identity)
        parts.append(_pack_k(wsg_bf[:, c * ISH:(c + 1) * ISH]))
        parts.append(_pack_k(wsu_bf[:, c * ISH:(c + 1) * ISH]))
        parts.append(wsd_sl.reshape(ISC, 128, D).transpose(1, 0, 2)
                     .reshape(128, ISC * D))
        for s, (e, idx, _) in enumerate(per_core[c]):
            parts.append(expert_w(e))
        din = np.ascontiguousarray(np.concatenate(parts, axis=1))
        assert din.shape[1] == off["total"]
        in_maps.append({"din": din})
    return in_maps, per_core, n2, n1


def kernel(**inputs):
    in_maps, per_core, n2, n1 = build_in_maps(inputs)
    nc = get_program(n2, n1)
    res = run_bass_kernel_spmd(nc, in_maps, core_ids=list(range(NCORES)))

    slot_caps = [256] * n2 + [128] * n1
    order = _slot_order(n2, n1)
    out = np.zeros((T, D), np.float32)
    for c in range(NCORES):
        r = res.results[c]["yo"].astype(np.float32)
        out += r[:8].reshape(T, D)
        row = 8
        for s in order:
            e, idx, wv = per_core[c][s]
            cap = slot_caps[s]
            ntt = cap // 128
            y = r[row:row + ntt].reshape(cap, D)
            row += ntt
            if len(idx):
                out[idx] += wv[:, None] * y[:len(idx)]
    return out.astype(np.float32)



# revision 52
# speedup vs baseline: 1.2055x; 1.1135x over previous
"""Expert-parallel DeepseekV2 MoE kernel for 8 Trainium2 NeuronCores, v7.

vs v3:
  - ALL inputs in one [128, N] bf16 tensor (per-iteration overhead through
    this exec path is ~29us per argument, so argument count is minimized).
  - Shared m1 computes hs^T directly (wsg/wsu tiles stationary, x^T moving)
    instead of m1-then-PE-transpose: fewer PE ops, fewer DVE copies.

Layout of din columns:
  [ xt_packed (KT*T) | xg slot 0..n (KT*cap each) | ident (128)
  | wsg (KT*ISH) | wsu (KT*ISH) | wsd (ISC*D)
  | slot 0: wg 16*I | wu 16*I | wd IT*D | slot 1: ... ]
Output rows: [ys tile 0..7 | ye slot tiles in order].
"""

import numpy as np
import ml_dtypes

import concourse.bass as bass
import concourse.tile as tile
from concourse import bacc, mybir
from concourse.bass_utils import run_bass_kernel_spmd

T, D = 1024, 2048
E, I = 32, 1408
TOPK = 6
N_GROUP, TOPK_GROUP = 8, 3
ROUTED_SCALE = 2.5
SHARED_I = 2 * I

NCORES = 8
ISH = SHARED_I // NCORES   # 352
KT = D // 128              # 16
IT = I // 128              # 11
ISC = 3
IS_SZ = [128, 128, ISH - 256]
WSLOT = 16 * I + 16 * I + IT * D   # 67584 cols per routed slot

F32 = mybir.dt.float32
BF16 = mybir.dt.bfloat16
SILU = mybir.ActivationFunctionType.Silu
COPYF = mybir.ActivationFunctionType.Copy
BF = ml_dtypes.bfloat16

_PROGRAM_CACHE = {}


def _col_layout(slot_caps):
    off = {}
    o = 0
    off["xt"] = o; o += KT * T
    off["xg"] = []
    for c in slot_caps:
        off["xg"].append(o); o += KT * c
    off["ident"] = o; o += 128
    off["wsg"] = o; o += KT * ISH
    off["wsu"] = o; o += KT * ISH
    off["wsd"] = o; o += ISC * D
    off["wr"] = []
    for _ in slot_caps:
        off["wr"].append(o); o += WSLOT
    off["total"] = o
    return off


def _build_program(n2, n1, reps=1):
    nc = bacc.Bacc("TRN2", target_bir_lowering=False, debug=False)

    slot_caps = [256] * n2 + [128] * n1
    ntt_total = sum(c // 128 for c in slot_caps)
    off = _col_layout(slot_caps)
    order = _slot_order(n2, n1)

    din = nc.dram_tensor("din", [128, off["total"]], BF16,
                         kind="ExternalInput").ap()
    yo = nc.dram_tensor("yo", [8 + ntt_total, 128, D], BF16,
                        kind="ExternalOutput").ap()

    from contextlib import ExitStack
    with tile.TileContext(nc) as tc, \
         tc.tile_pool(name="psum", bufs=8, space="PSUM") as psum, \
         tc.tile_pool(name="shres", bufs=1) as shres, \
         tc.tile_pool(name="hspool", bufs=3) as hspool, \
         tc.tile_pool(name="yspool", bufs=2) as yspool, \
         tc.tile_pool(name="xgpool", bufs=2) as xgpool, \
         tc.tile_pool(name="wpool", bufs=5) as wpool, \
         tc.tile_pool(name="wdpool", bufs=3) as wdpool, \
         tc.tile_pool(name="hgpool", bufs=2) as hgpool, \
         tc.tile_pool(name="hpool", bufs=2) as hpool, \
         tc.tile_pool(name="htpool", bufs=2) as htpool, \
         tc.tile_pool(name="ypool", bufs=2) as ypool, \
         ExitStack() as _rep_ctx:

        if reps > 1:
            _rep_ctx.enter_context(tc.For_i(0, reps, name="rep"))

        # Interleave xt / wsg / wsu chunks in shared-m1 consumption order
        # (kc ascending) so the first matmul waits on ~1.4MB, not ~8.4MB.
        xt0_sb = shres.tile([128, KT * 512], BF16, tag="xt0")
        xt1_sb = shres.tile([128, KT * 512], BF16, tag="xt1")
        wsg_sb = shres.tile([128, KT * ISH], BF16, tag="wsg")
        wsu_sb = shres.tile([128, KT * ISH], BF16, tag="wsu")
        HW = KT * 512
        _ingrp = [(0, 1), (1, 1), (2, 2), (4, 4), (8, 8)]
        for g0, gn in _ingrp:
            nc.sync.dma_start(out=wsg_sb[:, g0 * ISH:(g0 + gn) * ISH],
                              in_=din[:, off["wsg"] + g0 * ISH:
                                       off["wsg"] + (g0 + gn) * ISH])
            nc.sync.dma_start(out=xt0_sb[:, g0 * 512:(g0 + gn) * 512],
                              in_=din[:, g0 * 512:(g0 + gn) * 512])
            nc.sync.dma_start(out=wsu_sb[:, g0 * ISH:(g0 + gn) * ISH],
                              in_=din[:, off["wsu"] + g0 * ISH:
                                       off["wsu"] + (g0 + gn) * ISH])
            if g0 == 4:
                nc.sync.dma_start(out=xt1_sb[:, :HW // 2],
                                  in_=din[:, HW:HW + HW // 2])
        nc.sync.dma_start(out=xt1_sb[:, HW // 2:],
                          in_=din[:, HW + HW // 2:2 * HW])
        wsd_sb = shres.tile([128, ISC * D], BF16, tag="wsd")
        id_sb = shres.tile([128, 128], BF16, tag="ident")

        def _load_wsd_ident():
            nc.sync.dma_start(out=wsd_sb[:],
                              in_=din[:, off["wsd"]:off["wsd"] + ISC * D])
            nc.sync.dma_start(out=id_sb[:],
                              in_=din[:, off["ident"]:off["ident"] + 128])

        hsT_sb = shres.tile([128, ISC, T], BF16, tag="hsT")

        def m1_cmul(c, half, pg, pu):
            sz = IS_SZ[c]
            tsl = slice(half * 512, (half + 1) * 512)
            hsg = hspool.tile([128, 512], BF16, tag="hsg")
            nc.scalar.activation(hsg[:sz, :], pg[:sz, :], SILU)
            nc.vector.tensor_mul(hsT_sb[:sz, c, tsl], pu[:sz, :], hsg[:sz, :])

        # shared m1, half 0: streamed kc-outer at the head (consumes input
        # chunks as they arrive).
        def m1_half_stream(half):
            pg = {c: psum.tile([128, 512], F32, tag="ps", name=f"sg{half}_{c}")
                  for c in range(ISC)}
            pu = {c: psum.tile([128, 512], F32, tag="ps", name=f"su{half}_{c}")
                  for c in range(ISC)}
            xts = xt0_sb if half == 0 else xt1_sb
            for kc in range(KT):
                xmov = xts[:, kc * 512:(kc + 1) * 512]
                for c in range(ISC):
                    sz = IS_SZ[c]
                    nc.tensor.matmul(
                        pg[c][:sz, :], wsg_sb[:, kc * ISH + c * 128:
                                              kc * ISH + c * 128 + sz],
                        xmov, start=(kc == 0), stop=(kc == KT - 1))
                    nc.tensor.matmul(
                        pu[c][:sz, :], wsu_sb[:, kc * ISH + c * 128:
                                              kc * ISH + c * 128 + sz],
                        xmov, start=(kc == 0), stop=(kc == KT - 1))
            for c in range(ISC):
                m1_cmul(c, half, pg[c], pu[c])

        # shared m2 half-piece: 2 of 4 q-quarters of one 128-token tile.
        # 6 matmuls, 2 psum banks, ~2.6us of DMA-free PE work.
        def m2_half(tt, qp):
            qs = (0, 1) if qp == 0 else (2, 3)
            psy = {q: psum.tile([128, 512], F32, tag="ps", name=f"sy{tt}_{q}")
                   for q in qs}
            for c in range(ISC):
                sz = IS_SZ[c]
                for q in qs:
                    nc.tensor.matmul(
                        psy[q][:],
                        hsT_sb[:sz, c, tt * 128:(tt + 1) * 128],
                        wsd_sb[:sz, c * D + q * 512: c * D + (q + 1) * 512],
                        start=(c == 0), stop=(c == ISC - 1))
            ysb = yspool.tile([128, 1024], BF16, tag="ysb")
            for k, q in enumerate(qs):
                nc.vector.tensor_copy(ysb[:, k * 512:(k + 1) * 512],
                                      psy[q][:])
            nc.gpsimd.dma_start(out=yo[tt, :, qp * 1024:(qp + 1) * 1024],
                                in_=ysb[:])

        m1_half_stream(0)

        # shared m1 half-1 sub-units: 16 matmuls (3.4us) of DMA-free PE work
        # each; g and u of the same c-block may sit at different hooks (the
        # psum accumulator is carried between them).
        m1_pg = {}

        def m1_sub_g(c):
            sz = IS_SZ[c]
            pg = psum.tile([128, 512], F32, tag="ps", name=f"sg1_{c}")
            for kc in range(KT):
                nc.tensor.matmul(
                    pg[:sz, :], wsg_sb[:, kc * ISH + c * 128:
                                       kc * ISH + c * 128 + sz],
                    xt1_sb[:, kc * 512:(kc + 1) * 512],
                    start=(kc == 0), stop=(kc == KT - 1))
            m1_pg[c] = pg

        def m1_sub_u(c):
            sz = IS_SZ[c]
            pu = psum.tile([128, 512], F32, tag="ps", name=f"su1_{c}")
            for kc in range(KT):
                nc.tensor.matmul(
                    pu[:sz, :], wsu_sb[:, kc * ISH + c * 128:
                                       kc * ISH + c * 128 + sz],
                    xt1_sb[:, kc * 512:(kc + 1) * 512],
                    start=(kc == 0), stop=(kc == KT - 1))
            m1_cmul(c, 1, m1_pg.pop(c), pu)

        order = _slot_order(n2, n1)

        # ---- filler plan: DMA-free shared-expert units at the measured
        # stall points. Units: ("g",c)/("u",c) m1 sub-units (3.4us),
        # ("h",tt,qp) m2 half-pieces (1.3us), ("p",tt) full pieces (2.6us).
        fill_plan = {}
        if n2 == 3 and n1 == 2:
            fill_plan = {
                (0, "head"): [("g", 0), ("u", 0)],
                (0, "uend"): [("h", 0, 0)],
                (0, "m2start"): [("h", 0, 1)],
                (1, "gstart"): [("h", 1, 0)],
                (1, "uend"): [("h", 1, 1)],
                (1, "m2start"): [("h", 2, 0)],
                (2, "gstart"): [("h", 2, 1)],
                (2, "uend"): [("h", 3, 0)],
                (2, "m2start"): [("h", 3, 1)],
                (3, "gstart"): [("g", 1)],
                (3, "gmid"): [("u", 1)],
                (3, "gend"): [("g", 2), ("u", 2)],
                (3, "umid"): [("p", 4)],
                (4, "gstart"): [("p", 7)],
                (4, "gmid"): [("p", 5)],
                (4, "umid"): [("p", 6)],
            }
        else:
            fill_plan[(0, "head")] = [("g", c) for c in range(ISC)] + \
                                     [("u", c) for c in range(ISC)]
            hooks = []
            for pos, s in enumerate(order):
                if slot_caps[s] == 128:
                    hooks += [(pos, "gmid"), (pos, "gend"), (pos, "mmid")]
                elif pos > 0:
                    hooks.append((pos, "gstart"))
            if not hooks:
                hooks = [(len(order) - 1, "m2start")]
            for tt in range(8):
                fill_plan.setdefault(hooks[tt % len(hooks)], []).append(
                    ("p", tt))

        def emit_fill(pos, phase):
            for u in fill_plan.get((pos, phase), ()):
                if u[0] == "g":
                    m1_sub_g(u[1])
                elif u[0] == "u":
                    m1_sub_u(u[1])
                elif u[0] == "h":
                    m2_half(u[1], u[2])
                else:
                    m2_half(u[1], 0)
                    m2_half(u[1], 1)

        emit_fill(0, "head")

        next_yo_row = [8]
        for pos, s in enumerate(order):
            cap = slot_caps[s]
            ntt = cap // 128
            xgo = off["xg"][s]
            soff = off["wr"][s]
            xg_sb = xgpool.tile([128, KT * 256], BF16, tag="xg")
            nc.sync.dma_start(out=xg_sb[:, :KT * cap],
                              in_=din[:, xgo:xgo + KT * cap])

            def xg_lhs(kc, tt):
                return xg_sb[:, kc * cap + tt * 128: kc * cap + (tt + 1) * 128]

            emit_fill(pos, "gstart")
            hgs = {}
            hss = {}
            for mi, is_gate in ((0, True), (1, False)):
                moff = soff + mi * 16 * I
                ps = {(tt, j): psum.tile([128, 512], F32, tag="ps",
                                         name=f"p{s}_{mi}_{tt}_{j}")
                      for tt in range(ntt) for j in range(3)}
                for ch in range(4):
                    w_sb = wpool.tile([128, 4 * I], BF16, tag="wst",
                                      name="w_sb")
                    nc.sync.dma_start(
                        out=w_sb[:],
                        in_=din[:, moff + ch * 4 * I: moff + (ch + 1) * 4 * I])
                    if pos == 0 and mi == 1 and ch == 0:
                        _load_wsd_ident()
                    for a in range(4):
                        kc = ch * 4 + a
                        for tt in range(ntt):
                            lhs = xg_lhs(kc, tt)
                            for j in range(3):
                                sz = 512 if j < 2 else I - 1024
                                nc.tensor.matmul(
                                    ps[(tt, j)][:, :sz], lhs,
                                    w_sb[:, a * I + j * 512:
                                         a * I + j * 512 + sz],
                                    start=(kc == 0), stop=(kc == KT - 1))
                    if ch == 1:
                        emit_fill(pos, "gmid" if is_gate else "umid")
                if is_gate:
                    for tt in range(ntt):
                        hgs[tt] = hgpool.tile([128, I], BF16, tag="hg",
                                              name=f"hg{tt}")
                else:
                    for tt in range(ntt):
                        hss[tt] = hpool.tile([128, I], BF16, tag="h",
                                             name=f"h{tt}")
                for j in range(3):
                    sz = 512 if j < 2 else I - 1024
                    for tt in range(ntt):
                        if is_gate:
                            nc.scalar.activation(
                                hgs[tt][:, j * 512:j * 512 + sz],
                                ps[(tt, j)][:, :sz], SILU)
                        else:
                            nc.vector.tensor_mul(
                                hss[tt][:, j * 512:j * 512 + sz],
                                ps[(tt, j)][:, :sz],
                                hgs[tt][:, j * 512:j * 512 + sz])
                emit_fill(pos, "gend" if is_gate else "uend")

            hts = {tt: htpool.tile([128, IT * 128], BF16, tag="ht",
                                   name=f"ht{tt}")
                   for tt in range(ntt)}
            for g0, gcnt in ((0, 4), (4, 4), (8, 3)):
                for tt in range(ntt):
                    pst = psum.tile([128, 512], BF16, tag="ps",
                                    name=f"t{s}_{tt}_{g0}")
                    for k in range(gcnt):
                        ic = g0 + k
                        nc.tensor.transpose(
                            pst[:, k * 128:(k + 1) * 128],
                            hss[tt][:, ic * 128:(ic + 1) * 128], id_sb[:])
                    nc.vector.tensor_copy(
                        hts[tt][:, g0 * 128:(g0 + gcnt) * 128],
                        pst[:, :gcnt * 128])

            emit_fill(pos, "m2start")
            wdoff = soff + 2 * 16 * I
            psy = {(tt, q): psum.tile([128, 512], F32, tag="ps",
                                      name=f"y{s}_{tt}_{q}")
                   for tt in range(ntt) for q in range(4)}
            ichunks = [(0, 2), (2, 2), (4, 2), (6, 2), (8, 2), (10, 1)]
            for nch, (i0, cnt) in enumerate(ichunks):
                wd_sb = wdpool.tile([128, 2 * D], BF16, tag="wdst")
                nc.sync.dma_start(
                    out=wd_sb[:, :cnt * D],
                    in_=din[:, wdoff + i0 * D: wdoff + (i0 + cnt) * D])
                for a in range(cnt):
                    i = i0 + a
                    for tt in range(ntt):
                        for q in range(4):
                            nc.tensor.matmul(
                                psy[(tt, q)][:],
                                hts[tt][:, i * 128:(i + 1) * 128],
                                wd_sb[:, a * D + q * 512:
                                      a * D + (q + 1) * 512],
                                start=(i == 0), stop=(i == IT - 1))
                if nch == 2:
                    emit_fill(pos, "mmid")
            last = pos == len(order) - 1
            for tt in range(ntt):
                ysb = ypool.tile([128, D], BF16, tag="ye_sb")
                for q in range(4):
                    nc.vector.tensor_copy(ysb[:, q * 512:(q + 1) * 512],
                                          psy[(tt, q)][:])
                    if last and q % 2 == 1:
                        nc.gpsimd.dma_start(
                            out=yo[next_yo_row[0], :,
                                   (q - 1) * 512:(q + 1) * 512],
                            in_=ysb[:, (q - 1) * 512:(q + 1) * 512])
                if not last:
                    nc.gpsimd.dma_start(out=yo[next_yo_row[0]], in_=ysb[:])
                next_yo_row[0] += 1

    nc.compile()
    return nc


def _slot_order(n2, n1):
    """2-tile slots first: their weight streams are lighter than their PE
    demand, so DMA banks lookahead into the weight pools; the 1-tile slots
    (DMA-heavy, PE-light) then run cushioned by that lead plus filler."""
    return list(range(n2 + n1))


def get_program(n2, n1, reps=1):
    key = (n2, n1, reps)
    if key not in _PROGRAM_CACHE:
        _PROGRAM_CACHE[key] = _build_program(n2, n1, reps)
    return _PROGRAM_CACHE[key]


def _route_numpy(x, gate_w, bias):
    logits = x @ gate_w
    scores = 1.0 / (1.0 + np.exp(-logits))
    sc = scores + bias[None, :]
    g = sc.reshape(-1, N_GROUP, E // N_GROUP)
    group_scores = np.sort(g, axis=-1)[..., -2:].sum(-1)
    gidx = np.argsort(-group_scores, axis=-1, kind="stable")[:, :TOPK_GROUP]
    gmask = np.zeros((x.shape[0], N_GROUP), np.bool_)
    np.put_along_axis(gmask, gidx, True, axis=-1)
    emask = np.repeat(gmask, E // N_GROUP, axis=-1)
    masked = np.where(emask, sc, -np.inf)
    topk_idx = np.argsort(-masked, axis=-1, kind="stable")[:, :TOPK]
    w = np.take_along_axis(scores, topk_idx, axis=-1)
    w = w / (w.sum(-1, keepdims=True) + 1e-20)
    return topk_idx, w


def _plan(topk_idx, topk_w):
    flat_e = topk_idx.ravel()
    flat_t = np.repeat(np.arange(topk_idx.shape[0]), TOPK)
    flat_w = (topk_w * ROUTED_SCALE).ravel().astype(np.float32)
    order = np.argsort(flat_e, kind="stable")
    sorted_t = flat_t[order]
    sorted_w = flat_w[order]
    counts = np.bincount(flat_e, minlength=E)
    offsets = np.concatenate([[0], np.cumsum(counts)])

    two_slots, one_slots = [], []
    for e in range(E):
        toks = sorted_t[offsets[e]:offsets[e + 1]]
        ws_ = sorted_w[offsets[e]:offsets[e + 1]]
        n = len(toks)
        if n == 0:
            continue
        pos = 0
        while n - pos > 128:
            two_slots.append((e, toks[pos:pos + 256], ws_[pos:pos + 256]))
            pos += 256
        if n - pos > 0:
            one_slots.append((e, toks[pos:], ws_[pos:]))

    best = None
    for a in range(9):
        for b in range(9):
            for e2 in range(8):
                for e1 in range(8):
                    if a > len(one_slots) or b > len(two_slots):
                        continue
                    t2 = len(two_slots) + a - b + e2
                    t1 = len(one_slots) - a + 2 * b + e1
                    if t2 % NCORES or t1 % NCORES or t2 + t1 == 0:
                        continue
                    cost = 3 * (a + 2 * e2 + e1) + 2 * (b + e2 + e1)
                    if best is None or cost < best[0]:
                        best = (cost, a, b, e2, e1)
    _, a, b, e2, e1 = best
    for _ in range(a):
        one_slots.sort(key=lambda s: len(s[1]))
        two_slots.append(one_slots.pop(0))
    for _ in range(b):
        two_slots.sort(key=lambda s: len(s[1]))
        e, tk, ws_ = two_slots.pop()
        one_slots.append((e, tk[:128], ws_[:128]))
        one_slots.append((e, tk[128:], ws_[128:]))
    empty = (0, np.empty(0, np.int64), np.empty(0, np.float32))
    for _ in range(e2):
        two_slots.append(empty)
    for _ in range(e1):
        one_slots.append(empty)

    n2 = len(two_slots) // NCORES
    n1 = len(one_slots) // NCORES
    per_core = [[] for _ in range(NCORES)]
    for si, s in enumerate(two_slots):
        per_core[si % NCORES].append(s)
    for si, s in enumerate(one_slots):
        per_core[si % NCORES].append(s)
    return per_core, n2, n1


def _pack_k(a):
    m = a.shape[1]
    return np.ascontiguousarray(
        a.reshape(KT, 128, m).transpose(1, 0, 2).reshape(128, KT * m))


def _pack_w_chunks(w):
    """[D, I] -> [128, 16*I]: 4-ktile chunks side by side."""
    return np.ascontiguousarray(
        w.reshape(4, 4, 128, I).transpose(2, 0, 1, 3).reshape(128, 16 * I))


def build_in_maps(inputs):
    x = np.asarray(inputs["hidden_states"], np.float32)
    gate_w = np.asarray(inputs["gate_w"], np.float32)
    bias = np.asarray(inputs["e_score_correction_bias"], np.float32)
    w_gate = np.asarray(inputs["w_gate"], np.float32)
    w_up = np.asarray(inputs["w_up"], np.float32)
    w_down = np.asarray(inputs["w_down"], np.float32)
    ws_gate = np.asarray(inputs["ws_gate"], np.float32)
    ws_up = np.asarray(inputs["ws_up"], np.float32)
    ws_down = np.asarray(inputs["ws_down"], np.float32)

    topk_idx, topk_w = _route_numpy(x, gate_w, bias)
    per_core, n2, n1 = _plan(topk_idx, topk_w)
    slot_caps = [256] * n2 + [128] * n1
    off = _col_layout(slot_caps)

    xt_bf = np.ascontiguousarray(x.T.astype(BF))
    _a3 = xt_bf.reshape(KT, 128, T)
    xt_packed = np.concatenate(
        [np.ascontiguousarray(_a3[:, :, h * 512:(h + 1) * 512]
                              .transpose(1, 0, 2).reshape(128, KT * 512))
         for h in (0, 1)], axis=1)
    wg_bf = w_gate.astype(BF)
    wu_bf = w_up.astype(BF)
    wd_bf = w_down.astype(BF)
    wcache = {}

    def expert_w(e):
        if e not in wcache:
            wcache[e] = np.concatenate([
                _pack_w_chunks(wg_bf[e]),
                _pack_w_chunks(wu_bf[e]),
                wd_bf[e].reshape(IT, 128, D).transpose(1, 0, 2)
                .reshape(128, IT * D)], axis=1)
        return wcache[e]

    wsg_bf = ws_gate.astype(BF)
    wsu_bf = ws_up.astype(BF)
    wsd_bf = ws_down.astype(BF)
    identity = np.eye(128, dtype=BF)

    in_maps = []
    for c in range(NCORES):
        wsd_sl = np.zeros((ISC * 128, D), BF)
        wsd_sl[:ISH] = wsd_bf[c * ISH:(c + 1) * ISH]
        parts = [xt_packed]
        for s, (e, idx, _) in enumerate(per_core[c]):
            cap = slot_caps[s]
            xg = np.zeros((D, cap), BF)
            if len(idx):
                xg[:, :len(idx)] = xt_bf[:, idx]
            parts.append(_pack_k(xg))
        parts.append(identity)
        parts.append(_pack_k(wsg_bf[:, c * ISH:(c + 1) * ISH]))
        parts.append(_pack_k(wsu_bf[:, c * ISH:(c + 1) * ISH]))
        parts.append(wsd_sl.reshape(ISC, 128, D).transpose(1, 0, 2)
                     .reshape(128, ISC * D))
        for s, (e, idx, _) in enumerate(per_core[c]):
            parts.append(expert_w(e))
        din = np.ascontiguousarray(np.concatenate(parts, axis=1))
        assert din.shape[1] == off["total"]
        in_maps.append({"din": din})
    return in_maps, per_core, n2, n1


def kernel(**inputs):
    in_maps, per_core, n2, n1 = build_in_maps(inputs)
    nc = get_program(n2, n1)
    res = run_bass_kernel_spmd(nc, in_maps, core_ids=list(range(NCORES)))

    slot_caps = [256] * n2 + [128] * n1
    order = _slot_order(n2, n1)
    out = np.zeros((T, D), np.float32)
    for c in range(NCORES):
        r = res.results[c]["yo"].astype(np.float32)
        out += r[:8].reshape(T, D)
        row = 8
        for s in order:
            e, idx, wv = per_core[c][s]
            cap = slot_caps[s]
            ntt = cap // 128
            y = r[row:row + ntt].reshape(cap, D)
            row += ntt
            if len(idx):
                out[idx] += wv[:, None] * y[:len(idx)]
    return out.astype(np.float32)

